# revision 11
# baseline (speedup 1.0000x reference)
"""Multi-head self-attention (B=4, N=2048, C=1024, H=16, D=64) on 8 NeuronCores.

Sharding: (batch, head-group) -> core.  Core i handles batch b = i // 2 and
heads hg = i % 2 (8 heads each).  Each core computes its 8 heads' attention and
a partial output projection; the host sums the two partials per batch element
and adds b_o.

Per-core device pipeline (all matmul inputs bf16, fp32 PSUM accumulation):
  xT [C, N] (x transposed on host)
  QT = (w_q.T @ x.T + b_q) stored [head-dims, N]   (d on partitions, head pair per 128)
  KT likewise;  V natural [N, head-dims] with a ones column per head (row sums)
  S^T[j, q] = K^T.T @ Q^T per head (keys on partitions)  ->  exp via ACT (scale 1/8)
  O^T[d, q] += Vpad.T @ P^T accumulated over j tiles; row 64 = softmax denominator
  normalize via reciprocal + ones-broadcast matmul, evict to OT [head-dims, N]
  out[q, :] = OT.T @ w_o  (partial; host adds pair + b_o)
"""

import sys
import numpy as np

sys.path.insert(0, "/opt/trn_rl_repo")

import ml_dtypes  # noqa: E402

B, N, C, H, D = 4, 2048, 1024, 16, 64
P = 128
NCORES = 8
HEADS_PER_CORE = H // 2  # 8
HD = HEADS_PER_CORE * D  # 512 head-dims per core

_cache = {}


def _build_nc(n=N, c=C, heads=HEADS_PER_CORE, d=D, qcn=512, num_devices=NCORES,
              dt_name="bfloat16", debug_dump=False):
    import concourse.bacc as bacc
    import concourse.tile as tile
    import concourse.mybir as mybir

    dt = getattr(mybir.dt, dt_name)
    f32 = mybir.dt.float32
    f32r = mybir.dt.float32r
    add_op = mybir.AluOpType.add
    Exp = mybir.ActivationFunctionType.Exp

    hd = heads * d
    CT = c // P            # qkv contraction tiles
    MT = hd // P           # head-pair tiles (2 heads per tile)
    QC = n // qcn          # query chunks
    NT = n // P            # sequence tiles (key/j tiles)
    OCN = min(512, c)      # oproj output column chunk
    OC = c // OCN
    KO = hd // P           # oproj contraction tiles
    scale = float(d) ** -0.5
    assert d == 64 and MT * P == hd and CT * P == c

    nc = bacc.Bacc("TRN2", target_bir_lowering=False, debug=False,
                   num_devices=num_devices)

    xT_d = nc.declare_dram_parameter("xT", [c, n], dt, isOutput=False)
    wq_d = nc.declare_dram_parameter("wq", [c, hd], dt, isOutput=False)
    wk_d = nc.declare_dram_parameter("wk", [c, hd], dt, isOutput=False)
    wv_d = nc.declare_dram_parameter("wv", [c, hd], dt, isOutput=False)
    wo_d = nc.declare_dram_parameter("wo", [hd, c], dt, isOutput=False)
    bq_d = nc.declare_dram_parameter("bq", [MT, P], f32, isOutput=False)
    bk_d = nc.declare_dram_parameter("bk", [MT, P], f32, isOutput=False)
    bv_d = nc.declare_dram_parameter("bv", [P, hd], f32, isOutput=False)
    out_d = nc.declare_dram_parameter("out", [n, c], f32, isOutput=True)
    if debug_dump:
        dbg_qt = nc.declare_dram_parameter("dbg_qt", [P, MT, n], dt, isOutput=True)
        dbg_kt = nc.declare_dram_parameter("dbg_kt", [P, MT, n], dt, isOutput=True)
        dbg_vp = nc.declare_dram_parameter("dbg_vp", [P, NT, heads * (d + 1)], dt, isOutput=True)
        dbg_ot = nc.declare_dram_parameter("dbg_ot", [P, KO, n], dt, isOutput=True)
        dbg_bvb = nc.declare_dram_parameter("dbg_bvb", [P, hd], f32, isOutput=True)

    with tile.TileContext(nc) as tc:
        with tc.tile_pool(name="singles", bufs=1) as singles, \
             tc.tile_pool(name="pt_pool", bufs=4) as pt_pool, \
             tc.tile_pool(name="norm_pool", bufs=4) as norm_pool, \
             tc.tile_pool(name="evict_pool", bufs=4) as evict_pool, \
             tc.tile_pool(name="ps_mm", bufs=4, space="PSUM") as ps_mm, \
             tc.tile_pool(name="ps_o", bufs=2, space="PSUM") as ps_o:

            # ---- resident tensors -------------------------------------
            xt = singles.tile([P, CT, n], dt)
            wqt = singles.tile([P, CT, hd], dt)
            wkt = singles.tile([P, CT, hd], dt)
            wvt = singles.tile([P, CT, hd], dt)
            wot = singles.tile([P, KO, c], dt)
            bqt = singles.tile([P, MT], f32)
            bkt = singles.tile([P, MT], f32)
            bvb = singles.tile([P, hd], f32)
            qt_t = singles.tile([P, MT, n], dt)
            kt_t = singles.tile([P, MT, n], dt)
            vpad = singles.tile([P, NT, heads * (d + 1)], dt)
            ot_t = singles.tile([P, KO, n], dt)
            ones1_f = singles.tile([1, 64], f32)

            for ct in range(CT):
                cs = slice(ct * P, (ct + 1) * P)
                nc.sync.dma_start(out=xt[:, ct, :], in_=xT_d[cs, :])
                nc.sync.dma_start(out=wqt[:, ct, :], in_=wq_d[cs, :])
                nc.sync.dma_start(out=wkt[:, ct, :], in_=wk_d[cs, :])
                nc.sync.dma_start(out=wvt[:, ct, :], in_=wv_d[cs, :])
            for ko in range(KO):
                nc.sync.dma_start(out=wot[:, ko, :], in_=wo_d[ko * P:(ko + 1) * P, :])
            nc.sync.dma_start(out=bqt, in_=bq_d[:].rearrange("t p -> p t"))
            nc.sync.dma_start(out=bkt, in_=bk_d[:].rearrange("t p -> p t"))
            nc.sync.dma_start(out=bvb, in_=bv_d[:, :])
            nc.vector.memset(ones1_f, 1.0)
            ones1 = ones1_f.bitcast(f32r)
            nc.vector.memset(vpad, 1.0)  # ones columns; V parts overwritten

            # ---- phase 1: projections --------------------------------
            # V: [n, hd] with ones col per head
            for nt in range(NT):
                psv = ps_mm.tile([P, hd], f32, tag="mm", name=f"psv{nt}")
                for ct in range(CT):
                    nc.tensor.matmul(psv, xt[:, ct, nt * P:(nt + 1) * P],
                                     wvt[:, ct, :],
                                     start=(ct == 0), stop=(ct == CT - 1))
                vtgt = vpad[:, nt, :].rearrange("p (h e) -> p h e", e=d + 1)[:, :, :d]
                nc.vector.tensor_add(
                    vtgt,
                    psv.rearrange("p (h e) -> p h e", e=d),
                    bvb.rearrange("p (h e) -> p h e", e=d),
                )

            # KT then QT: [head-dims, n]
            for (w_t, b_t, dst) in ((wkt, bkt, kt_t), (wqt, bqt, qt_t)):
                for qc in range(QC):
                    qs = slice(qc * qcn, (qc + 1) * qcn)
                    for mt in range(MT):
                        psp = ps_mm.tile([P, qcn], f32, tag="mm", name=f"psp{qc}_{mt}")
                        for ct in range(CT):
                            nc.tensor.matmul(psp, w_t[:, ct, mt * P:(mt + 1) * P],
                                             xt[:, ct, qs],
                                             start=(ct == 0), stop=(ct == CT - 1))
                        nc.vector.tensor_scalar(
                            out=dst[:, mt, qs], in0=psp,
                            scalar1=b_t[:, mt:mt + 1], scalar2=None, op0=add_op)

            # ---- phase 2+3: attention, then oproj per q chunk ---------
            for qc in range(QC):
                qs = slice(qc * qcn, (qc + 1) * qcn)
                for mt in range(MT):
                    poA = ps_o.tile([65, qcn], f32, tag="poA", name=f"poA{qc}_{mt}")
                    poB = ps_o.tile([65, qcn], f32, tag="poB", name=f"poB{qc}_{mt}")
                    for jt in range(NT):
                        js = slice(jt * P, (jt + 1) * P)
                        psA = ps_mm.tile([P, qcn], f32, tag="mm", name=f"psA{jt}")
                        psB = ps_mm.tile([P, qcn], f32, tag="mm", name=f"psB{jt}")
                        nc.tensor.matmul(psA, kt_t[0:64, mt, js], qt_t[0:64, mt, qs],
                                         start=True, stop=True)
                        nc.tensor.matmul(psB, kt_t[64:128, mt, js], qt_t[64:128, mt, qs],
                                         start=True, stop=True)
                        ptA = pt_pool.tile([P, qcn], dt, tag="pt", name=f"ptA{jt}")
                        ptB = pt_pool.tile([P, qcn], dt, tag="pt", name=f"ptB{jt}")
                        nc.scalar.activation(ptA, psA, Exp, scale=scale)
                        nc.scalar.activation(ptB, psB, Exp, scale=scale)
                        hA, hB = 2 * mt, 2 * mt + 1
                        nc.tensor.matmul(poA, vpad[:, jt, hA * (d + 1):(hA + 1) * (d + 1)],
                                         ptA, start=(jt == 0), stop=(jt == NT - 1))
                        nc.tensor.matmul(poB, vpad[:, jt, hB * (d + 1):(hB + 1) * (d + 1)],
                                         ptB, start=(jt == 0), stop=(jt == NT - 1))
                    # normalize + evict O^T
                    for side, po in (("A", poA), ("B", poB)):
                        rc = norm_pool.tile([1, qcn], f32r, tag="rc", name=f"rc{side}")
                        with nc.allow_low_precision(reason="f32r is 4-byte"):
                            nc.vector.reciprocal(rc, po[64:65, :])
                        pb = ps_mm.tile([64, qcn], f32, tag="mm", name=f"pb{side}")
                        nc.tensor.matmul(pb, ones1, rc, start=True, stop=True)
                        bsb = norm_pool.tile([64, qcn], f32, tag="bsb", name=f"bsb{side}")
                        nc.vector.tensor_copy(bsb, pb)
                        if side == "A":
                            nc.vector.tensor_mul(ot_t[0:64, mt, qs], po[0:64, :], bsb)
                        else:
                            tmpB = norm_pool.tile([64, qcn], dt, tag="tmpB", name="tmpB")
                            nc.vector.tensor_mul(tmpB, po[0:64, :], bsb)
                            nc.sync.dma_start(out=ot_t[64:128, mt, qs], in_=tmpB)

                # oproj for the q tiles of this chunk
                for qt_i in range(qc * (qcn // P), (qc + 1) * (qcn // P)):
                    ts_ = slice(qt_i * P, (qt_i + 1) * P)
                    for oc in range(OC):
                        ocs = slice(oc * OCN, (oc + 1) * OCN)
                        pso = ps_mm.tile([P, OCN], f32, tag="mm", name=f"pso{qt_i}_{oc}")
                        for ko in range(KO):
                            nc.tensor.matmul(pso, ot_t[:, ko, ts_], wot[:, ko, ocs],
                                             start=(ko == 0), stop=(ko == KO - 1))
                        st = evict_pool.tile([P, OCN], f32, tag="st", name=f"st{qt_i}_{oc}")
                        nc.vector.tensor_copy(st, pso)
                        nc.sync.dma_start(out=out_d[ts_, ocs], in_=st)

            if debug_dump:
                nc.sync.dma_start(out=dbg_qt[:], in_=qt_t)
                nc.sync.dma_start(out=dbg_kt[:], in_=kt_t)
                nc.sync.dma_start(out=dbg_vp[:], in_=vpad)
                nc.sync.dma_start(out=dbg_ot[:], in_=ot_t)
                nc.sync.dma_start(out=dbg_bvb[:], in_=bvb)

    nc.compile()
    return nc


def _get_runner():
    """Build nc once and return a cached callable in_maps -> list of out dicts.

    Replicates run_bass_kernel_spmd's axon/PJRT path (bass2jax) but keeps the
    jitted executable cached across kernel() invocations so the NEFF is
    compiled exactly once per process.
    """
    if "runner" in _cache:
        return _cache["runner"]

    import jax
    from jax.experimental.shard_map import shard_map
    from jax.sharding import Mesh, PartitionSpec
    import concourse.mybir as mybir
    from concourse.bass2jax import (_bass_exec_p, install_neuronx_cc_hook,
                                    partition_id_tensor)

    nc = _build_nc()
    _cache["nc"] = nc
    install_neuronx_cc_hook()

    partition_name = (nc.partition_id_tensor.name
                      if nc.partition_id_tensor else None)
    in_names, out_names, out_avals, zero_outs = [], [], [], []
    for alloc in nc.m.functions[0].allocations:
        if not isinstance(alloc, mybir.MemoryLocationSet):
            continue
        name = alloc.memorylocations[0].name
        if alloc.kind == "ExternalInput":
            if name != partition_name:
                in_names.append(name)
        elif alloc.kind == "ExternalOutput":
            out_names.append(name)
            shape = tuple(alloc.tensor_shape)
            np_dt = mybir.dt.np(alloc.dtype)
            out_avals.append(jax.core.ShapedArray(shape, np_dt))
            zero_outs.append(np.zeros(shape, np_dt))
    n_params = len(in_names)
    n_outs = len(out_avals)
    all_in_names = list(in_names) + list(out_names)
    if partition_name is not None:
        all_in_names.append(partition_name)

    def _body(*args):
        operands = list(args)
        if partition_name is not None:
            operands.append(partition_id_tensor())
        outs = _bass_exec_p.bind(
            *operands,
            out_avals=tuple(out_avals),
            in_names=tuple(all_in_names),
            out_names=tuple(out_names),
            lowering_input_output_aliases=(),
            sim_require_finite=True,
            sim_require_nnan=True,
            nc=nc,
        )
        return tuple(outs)

    devices = jax.devices()[:NCORES]
    assert len(devices) == NCORES, f"need {NCORES} cores, have {len(jax.devices())}"
    mesh = Mesh(np.asarray(devices), ("core",))
    in_specs = (PartitionSpec("core"),) * (n_params + n_outs)
    out_specs = (PartitionSpec("core"),) * n_outs
    sharded = jax.jit(
        shard_map(_body, mesh=mesh, in_specs=in_specs, out_specs=out_specs,
                  check_rep=False),
        donate_argnums=tuple(range(n_params, n_params + n_outs)),
        keep_unused=True,
    )

    def runner(in_maps):
        per_core = [[np.asarray(m[name]) for name in in_names] for m in in_maps]
        concat_in = [
            np.concatenate([per_core[cr][i] for cr in range(NCORES)], axis=0)
            for i in range(n_params)
        ] + [
            np.concatenate([z] * NCORES, axis=0) for z in zero_outs
        ]
        out_arrs = sharded(*concat_in)
        results = []
        for cr in range(NCORES):
            res = {}
            for i, name in enumerate(out_names):
                arr = np.asarray(out_arrs[i])
                rows = arr.shape[0] // NCORES
                res[name] = arr[cr * rows:(cr + 1) * rows]
            results.append(res)
        return results

    _cache["runner"] = runner
    _cache["meta"] = (in_names, out_names, out_avals, zero_outs, partition_name)
    return runner


def make_in_maps(x, w_q, b_q, w_k, b_k, w_v, b_v, w_o, b_o):
    bf16 = ml_dtypes.bfloat16
    in_maps = []
    for core in range(NCORES):
        b = core // 2
        hs = (core % 2) * HD
        in_maps.append({
            "xT": np.ascontiguousarray(x[b].T).astype(bf16),
            "wq": np.ascontiguousarray(w_q[:, hs:hs + HD]).astype(bf16),
            "wk": np.ascontiguousarray(w_k[:, hs:hs + HD]).astype(bf16),
            "wv": np.ascontiguousarray(w_v[:, hs:hs + HD]).astype(bf16),
            "wo": np.ascontiguousarray(w_o[hs:hs + HD, :]).astype(bf16),
            "bq": np.ascontiguousarray(b_q[hs:hs + HD].reshape(-1, P)).astype(np.float32),
            "bk": np.ascontiguousarray(b_k[hs:hs + HD].reshape(-1, P)).astype(np.float32),
            "bv": np.ascontiguousarray(np.broadcast_to(
                b_v[hs:hs + HD].astype(np.float32), (P, HD))),
        })
    return in_maps


def kernel(x, w_q, b_q, w_k, b_k, w_v, b_v, w_o, b_o):
    x = np.asarray(x)
    runner = _get_runner()
    in_maps = make_in_maps(x, w_q, b_q, w_k, b_k, w_v, b_v, w_o, b_o)
    results = runner(in_maps)
    out = np.empty((B, N, C), np.float32)
    bo = np.asarray(b_o, dtype=np.float32)
    for b in range(B):
        out[b] = results[2 * b]["out"] + results[2 * b + 1]["out"] + bo
    return out
